# revision 19
# baseline (speedup 1.0000x reference)
"""DIN attention layer kernel for Trainium2 (8 NeuronCores, data-parallel).

Reference computation (per batch b):
    att = [q, k, q-k, q*k]            # [T, 4M]
    h1  = relu(att @ W1 + b1)         # [T, D]
    h2  = relu(h1 @ W2 + b2)          # [T, D]
    s   = h2 @ w_score + b_score      # [T, 1]
    attn = softmax(s.T + mask * -1e9) # [1, T]
    out = attn @ values               # [1, D]

Key optimizations (all exact math, bf16 matmuls / fp32 accumulation):
  * Mask gather on host: masked tokens have attn = exp(-1e9) = 0 exactly,
    so only the ~half unmasked tokens are shipped/computed. Batches are
    sorted by unmasked count and dealt into 8 "slots" (one batch per core
    per slot) so each slot's static token length is the max over just its
    8 batches (~52-55% of T).
  * Algebraic fold of the concat matmul, with q absorbed on the host:
        att @ W1 = [q@(W1a+W1c) + b1]  (per-batch bias row, host fp32)
                 + k @ ((W1b - W1c) + q∘W1d)   (per-batch weight, K=256)
    so mm1 contracts 256 instead of 1024.
  * All layout work on host: keys pre-transposed to [m, t], weights
    pre-combined and pre-cast to bf16, biases pre-striped. Device DMAs
    are plain [128, X] copies; no on-device transposes or weight casts.
  * Rank-1 contractions (score, attn@values, softmax-sum, broadcast) run
    as 1-column matmuls with the big tensor on the stationary side: PE
    matmul cost is output-free-size cycles, so these are nearly free,
    and the softmax stays partition-major (no transpose bounce).
  * Softmax without max-subtraction (scores are O(1); masked lanes are
    exp(-1e9) = 0), per-partition sums fused into Exp via accum_out.
  * relu drains alternate between Scalar and Vector engines; attn@values
    for slot i is emitted near the end of slot i+1 so the PE never waits
    on the softmax chain.
"""

import numpy as np
import ml_dtypes

P = 128
B_FULL = 64    # total batches
T = 1024       # tokens
M = 256        # key feature dim
D = 1024       # hidden dim
MC = M // P    # key-feature chunks (2)
DC = D // P    # hidden chunks (8)
N_CORES = 8
SLOTS = 8      # batches per core
NEG = -1.0e9
BF = ml_dtypes.bfloat16

_built_cache = {}


def _splits(t):
    if t <= 512:
        return [(0, t)]
    return [(0, 512), (512, t - 512)]


def _build(sizes):
    import concourse.bass as bass  # noqa: F401
    import concourse.bacc as bacc
    import concourse.mybir as mybir
    import concourse.tile as tile
    from contextlib import ExitStack

    F32 = mybir.dt.float32
    BF16 = mybir.dt.bfloat16
    AF = mybir.ActivationFunctionType
    OP = mybir.AluOpType

    tos = [(t + P - 1) // P for t in sizes]
    T0, TO0 = sizes[0], tos[0]

    nc = bacc.Bacc("TRN2")
    kT_d = [nc.dram_tensor(f"kT{i}", [MC, P, sizes[i]], BF16, kind="ExternalInput").ap()
            for i in range(SLOTS)]
    w1e_d = [nc.dram_tensor(f"w1e{i}", [MC, P, D], BF16, kind="ExternalInput").ap()
             for i in range(SLOTS)]
    vals_d = [nc.dram_tensor(f"vals{i}", [tos[i] * P, D], BF16, kind="ExternalInput").ap()
              for i in range(SLOTS)]
    rt_d = nc.dram_tensor("rt", [SLOTS, P, DC], F32, kind="ExternalInput").ap()
    mask_d = nc.dram_tensor("maskg", [SLOTS, P, TO0], F32, kind="ExternalInput").ap()
    w2_d = nc.dram_tensor("W2", [DC, P, D], BF16, kind="ExternalInput").ap()
    b2_d = nc.dram_tensor("b2s", [P, DC], F32, kind="ExternalInput").ap()
    ws_d = nc.dram_tensor("wss", [P, DC], BF16, kind="ExternalInput").ap()
    out_d = nc.dram_tensor("out", [SLOTS, P, DC], F32, kind="ExternalOutput").ap()

    with tile.TileContext(nc) as tc, ExitStack() as ctx:
        cons = ctx.enter_context(tc.tile_pool(name="cons", bufs=1))
        ktp = ctx.enter_context(tc.tile_pool(name="ktp", bufs=2))
        w1p = ctx.enter_context(tc.tile_pool(name="w1p", bufs=2))
        rtp = ctx.enter_context(tc.tile_pool(name="rtp", bufs=2))
        mkp = ctx.enter_context(tc.tile_pool(name="mkp", bufs=2))
        vpool = ctx.enter_context(tc.tile_pool(name="vp", bufs=3))
        h1pool = ctx.enter_context(tc.tile_pool(name="h1p", bufs=2))
        # all 8 h2 chunks stay alive until the score matmuls (whose psum
        # accumulation groups must be contiguous — interleaved groups in
        # one PSUM bank corrupt results on HW)
        h2pool = ctx.enter_context(tc.tile_pool(name="h2p", bufs=8))
        small = ctx.enter_context(tc.tile_pool(name="small", bufs=2))
        psum_mm = ctx.enter_context(tc.tile_pool(name="psmm", bufs=6, space="PSUM"))
        psum_sm = ctx.enter_context(tc.tile_pool(name="pssm", bufs=2, space="PSUM"))

        # ---- slot-0 inputs first (fine-split so the PE can start ASAP) --
        def load_kt(i, kt, fine):
            ti = sizes[i]
            if fine:
                step = 144
                for c in range(MC):
                    for s0 in range(0, ti, step):
                        sl = min(step, ti - s0)
                        nc.sync.dma_start(kt[:, c, s0:s0 + sl], kT_d[i][c, :, s0:s0 + sl])
            else:
                for c in range(MC):
                    nc.sync.dma_start(kt[:, c, :ti], kT_d[i][c])

        def load_w1e(i, w1e, fine):
            for c in range(MC):
                if fine:
                    for j in range(DC):
                        nc.sync.dma_start(w1e[:, c, j * P:(j + 1) * P],
                                          w1e_d[i][c, :, j * P:(j + 1) * P])
                else:
                    nc.sync.dma_start(w1e[:, c, :], w1e_d[i][c])

        kt0 = ktp.tile([P, MC, T0], BF16, tag="kt")
        load_kt(0, kt0, fine=True)
        w1e0 = w1p.tile([P, MC, D], BF16, tag="w1e")
        load_w1e(0, w1e0, fine=True)
        rt0 = rtp.tile([P, DC], F32, tag="rt")
        nc.sync.dma_start(rt0, rt_d[0])

        # ---- one-time setup --------------------------------------------
        w2_sb = cons.tile([P, DC, D], BF16)
        for c in range(DC):
            for h in range(4):
                nc.sync.dma_start(w2_sb[:, c, h * 256:(h + 1) * 256],
                                  w2_d[c, :, h * 256:(h + 1) * 256])
        b2_sb = cons.tile([P, DC], F32)
        nc.sync.dma_start(b2_sb, b2_d)
        ws_sb = cons.tile([P, DC], BF16)
        nc.sync.dma_start(ws_sb, ws_d)
        ones_sb = cons.tile([P, P], BF16)
        nc.vector.memset(ones_sb, 1.0)
        # warm the PE p-state and the Scalar activation table while the
        # first slot's DMAs stream in (these overlap the DMA wait)
        warm_sb = cons.tile([P, 1], BF16)
        nc.scalar.activation(warm_sb, ones_sb[:, 0:1], AF.Exp)
        warm_ps = psum_sm.tile([P, 512], F32, tag="sp")
        for _ in range(64):
            nc.tensor.matmul(warm_ps[:, :P], ones_sb, ones_sb,
                             start=True, stop=True)
        # NaN guard: h2 tiles are read by the column-score matmuls on
        # garbage token columns (masked away later); make them finite
        for _ in range(8):
            t_ = h2pool.tile([P, TO0 * P], BF16, tag="H2")
            nc.vector.memset(t_, 0.0)

        # ---- per-slot pipeline -----------------------------------------
        carry = {}

        def emit_attn_values(i):
            st = carry.pop(i)
            to_i = st["to"]
            # total = sum over partitions of per-partition exp sums,
            # broadcast to all partitions via a ones[128,128] matmul
            tot_ps = st["sp"][:, 31:32]
            nc.tensor.matmul(tot_ps, ones_sb, st["sums"], start=True, stop=True)
            rec_b = small.tile([P, 1], F32, tag="rec")
            nc.vector.reciprocal(rec_b, tot_ps)
            # out^T[d] = sum_t attn[t] vals[t, d], one psum column per
            # d-chunk, vals chunk as stationary side (1-column matmuls)
            outT_ps = st["sp"][:, 16:16 + DC]
            for j in range(DC):
                for c2 in range(to_i):
                    nc.tensor.matmul(
                        outT_ps[:, j:j + 1],
                        st["vals"][:, c2, j * P:(j + 1) * P],
                        st["attn_t"][:, c2:c2 + 1],
                        start=(c2 == 0), stop=(c2 == to_i - 1),
                        skip_group_check=True,
                    )
            outT_sb = small.tile([P, DC], F32, tag="osb")
            nc.vector.tensor_scalar_mul(outT_sb, outT_ps, rec_b)
            nc.sync.dma_start(out_d[i], outT_sb)

        for i in range(SLOTS):
            ti, to_i = sizes[i], tos[i]
            sp = _splits(ti)

            if i == 0:
                kt, w1e, rt_t = kt0, w1e0, rt0
            else:
                kt = ktp.tile([P, MC, T0], BF16, tag="kt")
                load_kt(i, kt, fine=False)
                w1e = w1p.tile([P, MC, D], BF16, tag="w1e")
                load_w1e(i, w1e, fine=False)
                rt_t = rtp.tile([P, DC], F32, tag="rt")
                nc.sync.dma_start(rt_t, rt_d[i])
            # mm1: h1[d, t] = relu(w1e.T @ kT + rt)
            h1 = h1pool.tile([P, DC, T0], BF16, tag="H1")
            for j in range(DC):
                for (s0, sl) in sp:
                    ps = psum_mm.tile([P, 512], F32, tag="mm")
                    for c in range(MC):
                        nc.tensor.matmul(
                            ps[:, :sl], w1e[:, c, j * P:(j + 1) * P],
                            kt[:, c, s0:s0 + sl],
                            start=(c == 0), stop=(c == MC - 1),
                        )
                    if j % 2 == 0:
                        nc.scalar.activation(
                            h1[:, j, s0:s0 + sl], ps[:, :sl], AF.Relu,
                            bias=rt_t[:, j:j + 1], scale=1.0)
                    else:
                        nc.vector.tensor_scalar(
                            h1[:, j, s0:s0 + sl], ps[:, :sl],
                            rt_t[:, j:j + 1], 0.0, op0=OP.add, op1=OP.max)

            # mm2; sp also hosts this slot's score and attn@values psum
            # regions (disjoint columns; groups never interleave)
            sp_t = psum_sm.tile([P, 512], F32, tag="sp")
            sc_ps = sp_t[:, 0:TO0]

            h2_tiles = {}
            for j in range(DC):
                h2 = h2pool.tile([P, TO0 * P], BF16, tag="H2")
                h2_tiles[j] = h2
                for (s0, sl) in sp:
                    ps = psum_mm.tile([P, 512], F32, tag="mm")
                    for c in range(DC):
                        nc.tensor.matmul(
                            ps[:, :sl], w2_sb[:, c, j * P:(j + 1) * P],
                            h1[:, c, s0:s0 + sl],
                            start=(c == 0), stop=(c == DC - 1),
                        )
                    if j % 2 == 1:
                        nc.scalar.activation(
                            h2[:, s0:s0 + sl], ps[:, :sl], AF.Relu,
                            bias=b2_sb[:, j:j + 1], scale=1.0)
                    else:
                        nc.vector.tensor_scalar(
                            h2[:, s0:s0 + sl], ps[:, :sl],
                            b2_sb[:, j:j + 1], 0.0, op0=OP.add, op1=OP.max)

            # vals/mask are only needed by attn@values (next slot) — load
            # them on the gpsimd queue pool, emitted late, so they never
            # delay the next slot's critical kt/w1e DMAs
            mask_t = mkp.tile([P, TO0], F32, tag="mask")
            nc.gpsimd.dma_start(mask_t[:, :to_i], mask_d[i, :, :to_i])
            vals_t = vpool.tile([P, TO0, D], BF16, tag="vals")
            for c2 in range(to_i):
                nc.gpsimd.dma_start(vals_t[:, c2, :], vals_d[i][c2 * P:(c2 + 1) * P])

            # deferred attn@values for the previous slot sits here, giving
            # the last h2 relu time to drain before its score matmuls
            if i > 0:
                emit_attn_values(i - 1)

            # column-form score: one psum column per token chunk, each an
            # uninterrupted 8-matmul accumulation group (1-column each)
            for tc in range(to_i):
                for j in range(DC):
                    nc.tensor.matmul(
                        sc_ps[:, tc:tc + 1],
                        h2_tiles[j][:, tc * P:(tc + 1) * P],
                        ws_sb[:, j:j + 1],
                        start=(j == 0), stop=(j == DC - 1),
                        skip_group_check=True,
                    )
            h2_tiles.clear()

            # partition-major softmax: lanes with mask=1 (incl. padding)
            # get -1e9 -> exp underflows to exactly 0
            attn_in = small.tile([P, TO0], F32, tag="attn_in")
            nc.vector.scalar_tensor_tensor(
                attn_in[:, :to_i], in0=mask_t[:, :to_i], scalar=NEG,
                in1=sc_ps[:, :to_i], op0=OP.mult, op1=OP.add)
            attn_t = small.tile([P, TO0], BF16, tag="attn")
            sums = small.tile([P, 1], F32, tag="sums")
            nc.scalar.activation(attn_t[:, :to_i], attn_in[:, :to_i], AF.Exp,
                                 accum_out=sums)
            # bf16 copy for the ones-matmul total reduce (keeps the program
            # free of fp32 matmuls, which slow the whole PE down)
            sums_bf = small.tile([P, 1], BF16, tag="sumsb")
            nc.vector.tensor_copy(sums_bf, sums)
            carry[i] = {"attn_t": attn_t, "vals": vals_t, "sums": sums_bf,
                        "to": to_i, "sp": sp_t}

        emit_attn_values(SLOTS - 1)

    nc.compile()
    return nc


def _get_built(sizes):
    nc = _built_cache.get(sizes)
    if nc is None:
        nc = _build(sizes)
        _built_cache[sizes] = nc
    return nc


def prepare(query, keys, values, mask, W1, b1, W2, b2, w_score, b_score=None):
    """Host-side preprocessing: gather unmasked tokens, fold q into the
    layer-1 weights/bias, pre-stripe/pre-cast everything to device layout.
    Returns (sizes, in_maps, order)."""
    q = np.asarray(query, np.float32).reshape(B_FULL, M)
    keys = np.asarray(keys, np.float32).reshape(B_FULL, T, M)
    values = np.asarray(values, np.float32).reshape(B_FULL, T, D)
    mask = np.asarray(mask, np.float32).reshape(B_FULL, T)
    W1 = np.asarray(W1, np.float32)
    b1 = np.asarray(b1, np.float32)
    W2 = np.asarray(W2, np.float32)
    b2 = np.asarray(b2, np.float32)
    ws = np.asarray(w_score, np.float32).reshape(D)

    unm = mask == 0.0
    counts = unm.sum(1)
    order = np.argsort(-counts, kind="stable")
    sizes = []
    for i in range(SLOTS):
        mx = int(counts[order[i * N_CORES]])
        sizes.append(max(8, min(T, ((mx + 7) // 8) * 8)))
    sizes = tuple(sizes)
    tos = [(t + P - 1) // P for t in sizes]
    TO0 = tos[0]

    W1qc = W1[0:M] + W1[2 * M:3 * M]
    W1bc = W1[M:2 * M] - W1[2 * M:3 * M]
    W1d = W1[3 * M:4 * M]
    rt_all = (q @ W1qc + b1).astype(np.float32)           # [64, D]

    W2h = np.ascontiguousarray(W2.reshape(DC, P, D).astype(BF))
    b2s = np.ascontiguousarray(b2.reshape(DC, P).T)
    wss = np.ascontiguousarray(ws.reshape(DC, P).T.astype(BF))

    in_maps = []
    for c in range(N_CORES):
        im = {"W2": W2h, "b2s": b2s, "wss": wss}
        rt_core = np.zeros((SLOTS, P, DC), np.float32)
        maskg = np.ones((SLOTS, P, TO0), np.float32)
        for i in range(SLOTS):
            b = int(order[i * N_CORES + c])
            ti, pi = sizes[i], tos[i] * P
            idx = np.nonzero(unm[b])[0]
            n = len(idx)
            kt = np.zeros((MC, P, ti), BF)
            kt[:, :, :n] = keys[b, idx].T.reshape(MC, P, n).astype(BF)
            im[f"kT{i}"] = kt
            im[f"w1e{i}"] = np.ascontiguousarray(
                (W1bc + q[b][:, None] * W1d).reshape(MC, P, D).astype(BF))
            va = np.zeros((pi, D), BF)
            va[:n] = values[b, idx].astype(BF)
            im[f"vals{i}"] = va
            rt_core[i] = rt_all[b].reshape(DC, P).T
            mg = np.ones(tos[i] * P, np.float32)
            mg[:n] = 0.0
            maskg[i, :, :tos[i]] = mg.reshape(tos[i], P).T
        im["rt"] = rt_core
        im["maskg"] = maskg
        in_maps.append(im)
    return sizes, in_maps, order


def gather_out(results, order):
    out = np.zeros((B_FULL, D), np.float32)
    for c in range(N_CORES):
        o = np.asarray(results[c]["out"], np.float32)   # [SLOTS, P, DC]
        for i in range(SLOTS):
            out[order[i * N_CORES + c]] = o[i].T.reshape(D)
    return out.reshape(B_FULL, 1, D)


def kernel(query, keys, values, mask, W1, b1, W2, b2, w_score, b_score):
    """Full-input entry point: shards over 8 NeuronCores, returns [64, 1, D]."""
    from concourse.bass_utils import run_bass_kernel_spmd

    sizes, in_maps, order = prepare(query, keys, values, mask,
                                    W1, b1, W2, b2, w_score)
    nc = _get_built(sizes)
    res = run_bass_kernel_spmd(nc, in_maps, core_ids=list(range(N_CORES)))
    return gather_out(res.results, order)


# revision 20
# speedup vs baseline: 1.0013x; 1.0013x over previous
"""DIN attention layer kernel for Trainium2 (8 NeuronCores, data-parallel).

Reference computation (per batch b):
    att = [q, k, q-k, q*k]            # [T, 4M]
    h1  = relu(att @ W1 + b1)         # [T, D]
    h2  = relu(h1 @ W2 + b2)          # [T, D]
    s   = h2 @ w_score + b_score      # [T, 1]
    attn = softmax(s.T + mask * -1e9) # [1, T]
    out = attn @ values               # [1, D]

Key optimizations (all exact math, bf16 matmuls / fp32 accumulation):
  * Mask gather on host: masked tokens have attn = exp(-1e9) = 0 exactly,
    so only the ~half unmasked tokens are shipped/computed. Batches are
    sorted by unmasked count and dealt into 8 "slots" (one batch per core
    per slot) so each slot's static token length is the max over just its
    8 batches (~52-55% of T).
  * Algebraic fold of the concat matmul, with q absorbed on the host:
        att @ W1 = [q@(W1a+W1c) + b1]  (per-batch bias row, host fp32)
                 + k @ ((W1b - W1c) + q∘W1d)   (per-batch weight, K=256)
    so mm1 contracts 256 instead of 1024.
  * All layout work on host: keys pre-transposed to [m, t], weights
    pre-combined and pre-cast to bf16, biases pre-striped. Device DMAs
    are plain [128, X] copies; no on-device transposes or weight casts.
  * Rank-1 contractions (score, attn@values, softmax-sum, broadcast) run
    as 1-column matmuls with the big tensor on the stationary side: PE
    matmul cost is output-free-size cycles, so these are nearly free,
    and the softmax stays partition-major (no transpose bounce).
  * Softmax without max-subtraction (scores are O(1); masked lanes are
    exp(-1e9) = 0), per-partition sums fused into Exp via accum_out.
  * relu drains alternate between Scalar and Vector engines; attn@values
    for slot i is emitted near the end of slot i+1 so the PE never waits
    on the softmax chain.
"""

import numpy as np
import ml_dtypes

P = 128
B_FULL = 64    # total batches
T = 1024       # tokens
M = 256        # key feature dim
D = 1024       # hidden dim
MC = M // P    # key-feature chunks (2)
DC = D // P    # hidden chunks (8)
N_CORES = 8
SLOTS = 8      # batches per core
NEG = -1.0e9
BF = ml_dtypes.bfloat16

_built_cache = {}


def _splits(t):
    if t <= 512:
        return [(0, t)]
    return [(0, 512), (512, t - 512)]


def _build(sizes):
    import concourse.bass as bass  # noqa: F401
    import concourse.bacc as bacc
    import concourse.mybir as mybir
    import concourse.tile as tile
    from contextlib import ExitStack

    F32 = mybir.dt.float32
    BF16 = mybir.dt.bfloat16
    AF = mybir.ActivationFunctionType
    OP = mybir.AluOpType

    tos = [(t + P - 1) // P for t in sizes]
    T0, TO0 = sizes[0], tos[0]

    nc = bacc.Bacc("TRN2")
    kT_d = [nc.dram_tensor(f"kT{i}", [MC, P, sizes[i]], BF16, kind="ExternalInput").ap()
            for i in range(SLOTS)]
    w1e_d = [nc.dram_tensor(f"w1e{i}", [MC, P, D], BF16, kind="ExternalInput").ap()
             for i in range(SLOTS)]
    vals_d = [nc.dram_tensor(f"vals{i}", [tos[i] * P, D], BF16, kind="ExternalInput").ap()
              for i in range(SLOTS)]
    rt_d = nc.dram_tensor("rt", [SLOTS, P, DC], F32, kind="ExternalInput").ap()
    mask_d = nc.dram_tensor("maskg", [SLOTS, P, TO0], F32, kind="ExternalInput").ap()
    w2_d = nc.dram_tensor("W2", [DC, P, D], BF16, kind="ExternalInput").ap()
    b2_d = nc.dram_tensor("b2s", [P, DC], F32, kind="ExternalInput").ap()
    ws_d = nc.dram_tensor("wss", [P, DC], BF16, kind="ExternalInput").ap()
    out_d = nc.dram_tensor("out", [SLOTS, P, DC], F32, kind="ExternalOutput").ap()

    with tile.TileContext(nc) as tc, ExitStack() as ctx:
        cons = ctx.enter_context(tc.tile_pool(name="cons", bufs=1))
        ktp = ctx.enter_context(tc.tile_pool(name="ktp", bufs=2))
        w1p = ctx.enter_context(tc.tile_pool(name="w1p", bufs=2))
        rtp = ctx.enter_context(tc.tile_pool(name="rtp", bufs=2))
        mkp = ctx.enter_context(tc.tile_pool(name="mkp", bufs=2))
        vpool = ctx.enter_context(tc.tile_pool(name="vp", bufs=3))
        h1pool = ctx.enter_context(tc.tile_pool(name="h1p", bufs=2))
        # all 8 h2 chunks stay alive until the score matmuls (whose psum
        # accumulation groups must be contiguous — interleaved groups in
        # one PSUM bank corrupt results on HW)
        h2pool = ctx.enter_context(tc.tile_pool(name="h2p", bufs=8))
        small = ctx.enter_context(tc.tile_pool(name="small", bufs=2))
        psum_mm = ctx.enter_context(tc.tile_pool(name="psmm", bufs=6, space="PSUM"))
        psum_sm = ctx.enter_context(tc.tile_pool(name="pssm", bufs=2, space="PSUM"))

        # ---- slot-0 inputs first (fine-split so the PE can start ASAP) --
        def load_kt(i, kt, fine):
            ti = sizes[i]
            if fine:
                step = 144
                for c in range(MC):
                    for s0 in range(0, ti, step):
                        sl = min(step, ti - s0)
                        nc.sync.dma_start(kt[:, c, s0:s0 + sl], kT_d[i][c, :, s0:s0 + sl])
            else:
                for c in range(MC):
                    nc.sync.dma_start(kt[:, c, :ti], kT_d[i][c])

        def load_w1e(i, w1e, fine):
            for c in range(MC):
                if fine:
                    for j in range(DC):
                        nc.sync.dma_start(w1e[:, c, j * P:(j + 1) * P],
                                          w1e_d[i][c, :, j * P:(j + 1) * P])
                else:
                    nc.sync.dma_start(w1e[:, c, :], w1e_d[i][c])

        kt0 = ktp.tile([P, MC, T0], BF16, tag="kt")
        load_kt(0, kt0, fine=True)
        w1e0 = w1p.tile([P, MC, D], BF16, tag="w1e")
        load_w1e(0, w1e0, fine=True)
        rt0 = rtp.tile([P, DC], F32, tag="rt")
        nc.sync.dma_start(rt0, rt_d[0])

        # ---- one-time setup --------------------------------------------
        w2_sb = cons.tile([P, DC, D], BF16)
        for c in range(DC):
            for h in range(4):
                nc.sync.dma_start(w2_sb[:, c, h * 256:(h + 1) * 256],
                                  w2_d[c, :, h * 256:(h + 1) * 256])
        b2_sb = cons.tile([P, DC], F32)
        nc.sync.dma_start(b2_sb, b2_d)
        ws_sb = cons.tile([P, DC], BF16)
        nc.sync.dma_start(ws_sb, ws_d)
        ones_sb = cons.tile([P, P], BF16)
        nc.vector.memset(ones_sb, 1.0)
        # warm the Scalar activation table while the first slot's DMAs
        # stream in (the first activation otherwise pays ACT_TABLE_LOAD)
        warm_sb = cons.tile([P, 1], BF16)
        nc.scalar.activation(warm_sb, ones_sb[:, 0:1], AF.Exp)
        # NaN guard: h2 tiles are read by the column-score matmuls on
        # garbage token columns (masked away later); make them finite
        for _ in range(8):
            t_ = h2pool.tile([P, TO0 * P], BF16, tag="H2")
            nc.vector.memset(t_, 0.0)

        # ---- per-slot pipeline -----------------------------------------
        carry = {}

        def emit_attn_values(i):
            st = carry.pop(i)
            to_i = st["to"]
            # total = sum over partitions of per-partition exp sums,
            # broadcast to all partitions via a ones[128,128] matmul
            tot_ps = st["sp"][:, 31:32]
            nc.tensor.matmul(tot_ps, ones_sb, st["sums"], start=True, stop=True)
            rec_b = small.tile([P, 1], F32, tag="rec")
            nc.vector.reciprocal(rec_b, tot_ps)
            # out^T[d] = sum_t attn[t] vals[t, d], one psum column per
            # d-chunk, vals chunk as stationary side (1-column matmuls)
            outT_ps = st["sp"][:, 16:16 + DC]
            for j in range(DC):
                for c2 in range(to_i):
                    nc.tensor.matmul(
                        outT_ps[:, j:j + 1],
                        st["vals"][:, c2, j * P:(j + 1) * P],
                        st["attn_t"][:, c2:c2 + 1],
                        start=(c2 == 0), stop=(c2 == to_i - 1),
                        skip_group_check=True,
                    )
            outT_sb = small.tile([P, DC], F32, tag="osb")
            nc.vector.tensor_scalar_mul(outT_sb, outT_ps, rec_b)
            nc.sync.dma_start(out_d[i], outT_sb)

        for i in range(SLOTS):
            ti, to_i = sizes[i], tos[i]
            sp = _splits(ti)

            if i == 0:
                kt, w1e, rt_t = kt0, w1e0, rt0
            else:
                kt = ktp.tile([P, MC, T0], BF16, tag="kt")
                load_kt(i, kt, fine=False)
                w1e = w1p.tile([P, MC, D], BF16, tag="w1e")
                load_w1e(i, w1e, fine=False)
                rt_t = rtp.tile([P, DC], F32, tag="rt")
                nc.sync.dma_start(rt_t, rt_d[i])
            # mm1: h1[d, t] = relu(w1e.T @ kT + rt)
            h1 = h1pool.tile([P, DC, T0], BF16, tag="H1")
            for j in range(DC):
                for (s0, sl) in sp:
                    ps = psum_mm.tile([P, 512], F32, tag="mm")
                    for c in range(MC):
                        nc.tensor.matmul(
                            ps[:, :sl], w1e[:, c, j * P:(j + 1) * P],
                            kt[:, c, s0:s0 + sl],
                            start=(c == 0), stop=(c == MC - 1),
                        )
                    if j % 2 == 0:
                        nc.scalar.activation(
                            h1[:, j, s0:s0 + sl], ps[:, :sl], AF.Relu,
                            bias=rt_t[:, j:j + 1], scale=1.0)
                    else:
                        nc.vector.tensor_scalar(
                            h1[:, j, s0:s0 + sl], ps[:, :sl],
                            rt_t[:, j:j + 1], 0.0, op0=OP.add, op1=OP.max)

            # mm2; sp also hosts this slot's score and attn@values psum
            # regions (disjoint columns; groups never interleave)
            sp_t = psum_sm.tile([P, 512], F32, tag="sp")
            sc_ps = sp_t[:, 0:TO0]

            h2_tiles = {}
            for j in range(DC):
                h2 = h2pool.tile([P, TO0 * P], BF16, tag="H2")
                h2_tiles[j] = h2
                for (s0, sl) in sp:
                    ps = psum_mm.tile([P, 512], F32, tag="mm")
                    for c in range(DC):
                        nc.tensor.matmul(
                            ps[:, :sl], w2_sb[:, c, j * P:(j + 1) * P],
                            h1[:, c, s0:s0 + sl],
                            start=(c == 0), stop=(c == DC - 1),
                        )
                    if j % 2 == 1:
                        nc.scalar.activation(
                            h2[:, s0:s0 + sl], ps[:, :sl], AF.Relu,
                            bias=b2_sb[:, j:j + 1], scale=1.0)
                    else:
                        nc.vector.tensor_scalar(
                            h2[:, s0:s0 + sl], ps[:, :sl],
                            b2_sb[:, j:j + 1], 0.0, op0=OP.add, op1=OP.max)

            # vals/mask are only needed by attn@values (next slot) — load
            # them on the gpsimd queue pool, emitted late, so they never
            # delay the next slot's critical kt/w1e DMAs
            mask_t = mkp.tile([P, TO0], F32, tag="mask")
            nc.gpsimd.dma_start(mask_t[:, :to_i], mask_d[i, :, :to_i])
            vals_t = vpool.tile([P, TO0, D], BF16, tag="vals")
            for c2 in range(to_i):
                nc.gpsimd.dma_start(vals_t[:, c2, :], vals_d[i][c2 * P:(c2 + 1) * P])

            # deferred attn@values for the previous slot sits here, giving
            # the last h2 relu time to drain before its score matmuls
            if i > 0:
                emit_attn_values(i - 1)

            # column-form score: one psum column per token chunk, each an
            # uninterrupted 8-matmul accumulation group (1-column each)
            for tc in range(to_i):
                for j in range(DC):
                    nc.tensor.matmul(
                        sc_ps[:, tc:tc + 1],
                        h2_tiles[j][:, tc * P:(tc + 1) * P],
                        ws_sb[:, j:j + 1],
                        start=(j == 0), stop=(j == DC - 1),
                        skip_group_check=True,
                    )
            h2_tiles.clear()

            # partition-major softmax: lanes with mask=1 (incl. padding)
            # get -1e9 -> exp underflows to exactly 0
            attn_in = small.tile([P, TO0], F32, tag="attn_in")
            nc.vector.scalar_tensor_tensor(
                attn_in[:, :to_i], in0=mask_t[:, :to_i], scalar=NEG,
                in1=sc_ps[:, :to_i], op0=OP.mult, op1=OP.add)
            attn_t = small.tile([P, TO0], BF16, tag="attn")
            sums = small.tile([P, 1], F32, tag="sums")
            nc.scalar.activation(attn_t[:, :to_i], attn_in[:, :to_i], AF.Exp,
                                 accum_out=sums)
            # bf16 copy for the ones-matmul total reduce (keeps the program
            # free of fp32 matmuls, which slow the whole PE down)
            sums_bf = small.tile([P, 1], BF16, tag="sumsb")
            nc.vector.tensor_copy(sums_bf, sums)
            carry[i] = {"attn_t": attn_t, "vals": vals_t, "sums": sums_bf,
                        "to": to_i, "sp": sp_t}

        emit_attn_values(SLOTS - 1)

    nc.compile()
    return nc


def _get_built(sizes):
    nc = _built_cache.get(sizes)
    if nc is None:
        nc = _build(sizes)
        _built_cache[sizes] = nc
    return nc


def prepare(query, keys, values, mask, W1, b1, W2, b2, w_score, b_score=None):
    """Host-side preprocessing: gather unmasked tokens, fold q into the
    layer-1 weights/bias, pre-stripe/pre-cast everything to device layout.
    Returns (sizes, in_maps, order)."""
    q = np.asarray(query, np.float32).reshape(B_FULL, M)
    keys = np.asarray(keys, np.float32).reshape(B_FULL, T, M)
    values = np.asarray(values, np.float32).reshape(B_FULL, T, D)
    mask = np.asarray(mask, np.float32).reshape(B_FULL, T)
    W1 = np.asarray(W1, np.float32)
    b1 = np.asarray(b1, np.float32)
    W2 = np.asarray(W2, np.float32)
    b2 = np.asarray(b2, np.float32)
    ws = np.asarray(w_score, np.float32).reshape(D)

    unm = mask == 0.0
    counts = unm.sum(1)
    order = np.argsort(-counts, kind="stable")
    sizes = []
    for i in range(SLOTS):
        mx = int(counts[order[i * N_CORES]])
        sizes.append(max(8, min(T, ((mx + 7) // 8) * 8)))
    sizes = tuple(sizes)
    tos = [(t + P - 1) // P for t in sizes]
    TO0 = tos[0]

    W1qc = W1[0:M] + W1[2 * M:3 * M]
    W1bc = W1[M:2 * M] - W1[2 * M:3 * M]
    W1d = W1[3 * M:4 * M]
    rt_all = (q @ W1qc + b1).astype(np.float32)           # [64, D]

    W2h = np.ascontiguousarray(W2.reshape(DC, P, D).astype(BF))
    b2s = np.ascontiguousarray(b2.reshape(DC, P).T)
    wss = np.ascontiguousarray(ws.reshape(DC, P).T.astype(BF))

    in_maps = []
    for c in range(N_CORES):
        im = {"W2": W2h, "b2s": b2s, "wss": wss}
        rt_core = np.zeros((SLOTS, P, DC), np.float32)
        maskg = np.ones((SLOTS, P, TO0), np.float32)
        for i in range(SLOTS):
            b = int(order[i * N_CORES + c])
            ti, pi = sizes[i], tos[i] * P
            idx = np.nonzero(unm[b])[0]
            n = len(idx)
            kt = np.zeros((MC, P, ti), BF)
            kt[:, :, :n] = keys[b, idx].T.reshape(MC, P, n).astype(BF)
            im[f"kT{i}"] = kt
            im[f"w1e{i}"] = np.ascontiguousarray(
                (W1bc + q[b][:, None] * W1d).reshape(MC, P, D).astype(BF))
            va = np.zeros((pi, D), BF)
            va[:n] = values[b, idx].astype(BF)
            im[f"vals{i}"] = va
            rt_core[i] = rt_all[b].reshape(DC, P).T
            mg = np.ones(tos[i] * P, np.float32)
            mg[:n] = 0.0
            maskg[i, :, :tos[i]] = mg.reshape(tos[i], P).T
        im["rt"] = rt_core
        im["maskg"] = maskg
        in_maps.append(im)
    return sizes, in_maps, order


def gather_out(results, order):
    out = np.zeros((B_FULL, D), np.float32)
    for c in range(N_CORES):
        o = np.asarray(results[c]["out"], np.float32)   # [SLOTS, P, DC]
        for i in range(SLOTS):
            out[order[i * N_CORES + c]] = o[i].T.reshape(D)
    return out.reshape(B_FULL, 1, D)


def kernel(query, keys, values, mask, W1, b1, W2, b2, w_score, b_score):
    """Full-input entry point: shards over 8 NeuronCores, returns [64, 1, D]."""
    from concourse.bass_utils import run_bass_kernel_spmd

    sizes, in_maps, order = prepare(query, keys, values, mask,
                                    W1, b1, W2, b2, w_score)
    nc = _get_built(sizes)
    res = run_bass_kernel_spmd(nc, in_maps, core_ids=list(range(N_CORES)))
    return gather_out(res.results, order)
